# revision 34
# baseline (speedup 1.0000x reference)
"""Causal multi-head attention (B=2, S=2048, E=1024, H=16, D=64) on 8 trn2 NeuronCores.

Sharding: core c handles batch b = c // 4 and head group g = c % 4 (4 heads each).
Each core computes, for its batch and its 4 heads:
    q/k/v = x @ W[qkv][:, 256g:256g+256], causal attention, then the partial
    projection  out_heads @ Wp[256g:256g+256, :]  -> [2048, 1024].
Host gathers: out[b] = sum_g partial[b, g] + bp  (the "all-reduce" of the TP hint).

fp16 end-to-end (inputs cast on host; tolerance is 2e-2, fp16 lands ~7e-4).
fp8 was evaluated and rejected: plain e4m3 on any main-path operand measures
2.3-2.9e-2 end-to-end (over the gate), and exact hi/lo-split variants cost
>= fp16 because DoubleRow halves the output rows (M<=64).

  - x and wq/wk/wv arrive HOST-PREPACKED in the exact SBUF layouts so each
    is ONE dma_start with 4-8KB contiguous lines; s-chunk 0 is split across
    the two HWDGE queues (sync+scalar, ~180GB/s each).  dma_start costs
    ~1.5us fixed (trigger + sem), so fewer/bigger transfers win.
  - the PE p-state ramps 0.65 -> 1.2 -> 2.4 GHz only after ~3us of
    continuous busy; ~30 dummy ident transposes bridge the initial DMA wait
    so real work starts at full clock.
  - x is PE-transposed (1 cyc/row fp16) into xT e-chunks; startup psum
    drains are split vector/scalar and alternate fill/sT pools so the 2-buf
    ring WAR never gates the PE.
  - kT/qT are [head-pair, S] tiles with the pair's heads stacked at
    partition 0/64; scores use tile_position=(64h, 0).
  - v is stored per (s-tile, head) as a [128, 128] slab [ones64 | v64]; the
    PV matmul then emits the softmax denominator on partitions 0..63 and the
    numerator on 64..127, so normalization is reciprocal+mul on the vector
    engine (no partition_broadcast, no copies).
  - ALL attention groups (block, head-pair, j-tile) run as ONE flat software
    pipeline (attn_pipeline): the 2-group score lookahead crosses hp and
    block boundaries, norms are emitted inline right after each hp's last
    PV, and filler jobs (qk chains / v tiles / transposes / projection
    halves) are popped per group to hide the scalar exp stream (~1.05us per
    512-wide group, the steady-state pacer).
  - each score matmul's PSUM region starts 512-f32 aligned: two matmul groups
    packed into one PSUM bank at sub-bank offsets fail at runtime.
  - q-blocks run in order [0,384) [512,1024) [1024,1536) [1536,2048) [384,512):
    the last block is 128 q wide, so the end tail (norm+proj+DMA) is short.
    0b's groups are latency-bound and absorb the proj fillers; 2 jobs are
    reserved past the last pop so the PE has work while the final norm
    drains on Vector, and the tail projection interleaves its two psum
    chains hp-outer so the oT_all[0] halves run during norm(hp1).
"""

import os
import sys
import numpy as np

sys.path.insert(0, "/opt/trn_rl_repo")

import concourse.bass as bass
import concourse.bacc as bacc_mod
import concourse.mybir as mybir
import concourse.tile as tile
from concourse import library_config

F32 = mybir.dt.float32
F16 = mybir.dt.float16
P = 128

B = 2
S = 2048
E = 1024
NHEADS_TOTAL = 16
D = 64
N_CORES = 8
GROUPS = 4                        # head groups (tensor parallel)
HD = NHEADS_TOTAL * D // GROUPS   # 256 head-dims per core
NH = HD // D                      # heads per core (4)
NHP = HD // P                     # head pairs (2)
NST = S // P                      # s tiles (16)
NEC = E // P                      # e chunks (8)
NSC = S // 512                    # 512-wide s chunks (4)


def build_core_program(lower_isa=True):
    """One NeuronCore's program (SPMD: all 8 cores run this on different data)."""
    nc = bacc_mod.Bacc()
    # x and wq/wk/wv arrive HOST-PREPACKED in the exact SBUF layouts
    # (x: [p, sc, k, e], w: [p, ec, n]) so every DMA line is 4-8KB
    # contiguous and each tensor is ONE dma_start: the startup was
    # descriptor/trigger-latency bound (~1.5us fixed per dma_start,
    # 2x bandwidth penalty for <512B lines).
    x_d = nc.declare_dram_parameter("x", [P, NSC * 4096], F16, False)
    wq_d = nc.declare_dram_parameter("wq", [P, NEC * HD], F16, False)
    wk_d = nc.declare_dram_parameter("wk", [P, NEC * HD], F16, False)
    wv_d = nc.declare_dram_parameter("wv", [P, NEC * HD], F16, False)
    wp_d = nc.declare_dram_parameter("wp", [HD, E], F16, False)
    # identity comes in as data: building it with gpsimd memset+affine_select
    # would serialize the first PE transpose behind the ~10us gpsimd library
    # load DMA.
    id_d = nc.declare_dram_parameter("ident", [P, P], F16, False)
    y_d = nc.declare_dram_parameter("y", [S, E], F16, True)

    with tile.TileContext(nc) as tc:
        from contextlib import ExitStack
        with ExitStack() as ctx:
            persist = ctx.enter_context(tc.tile_pool(name="persist", bufs=1))

            ident = persist.tile([P, P], F16, tag="ident", name="ident")

            xT = [persist.tile([P, S], F16, tag=f"xT{ec}", name=f"xT{ec}")
                  for ec in range(NEC)]
            xn = [persist.tile([P, 4096], F16, tag=f"xn{sc}", name=f"xn{sc}")
                  for sc in range(NSC)]
            wsb = {nm: persist.tile([P, NEC * HD], F16, tag=nm, name=nm)
                   for nm in ("wq", "wk", "wv")}
            wp_sb = [persist.tile([P, E], F16, tag=f"wp{hp}", name=f"wp{hp}")
                     for hp in range(NHP)]
            qT = [persist.tile([P, S], F16, tag=f"qT{hp}", name=f"qT{hp}")
                  for hp in range(NHP)]
            kT = [persist.tile([P, S], F16, tag=f"kT{hp}", name=f"kT{hp}")
                  for hp in range(NHP)]
            # per (s-tile, head) slab [128, 128] = [ones 0:64 | v 64:128]
            v_ext = persist.tile([P, NST * NH * P], F16, tag="v_ext", name="v_ext")
            oT_all = [persist.tile([P, S], F16, tag=f"oT{hp}", name=f"oT{hp}")
                      for hp in range(NHP)]

            v_view = v_ext.rearrange("p (s h c) -> p s h c", s=NST, h=NH)
            nc.vector.memset(
                v_ext.rearrange("p (s c) -> p s c", s=NST * NH)[:, :, 0:D], 1.0)

            # ---------------- DMA issue ----------------
            # Prepacked loads with 4-8KB lines.  Only sync (SP) and scalar
            # (Activation) queues can trigger HWDGE; each sustains ~180GB/s,
            # so s-chunk 0 is split across BOTH queues and the rest is
            # deadline-ordered: sync [ident, x0a, x1], scalar [x0b, wk, wq,
            # wv, wp].  x chunks 2/3 are deferred (XNV filler jobs), split
            # across both queues.
            nc.sync.dma_start(out=ident[:], in_=id_d[:, :])
            def issue_xn(sc, split=True):
                mid = 4096 * sc + 2048
                if split:
                    nc.sync.dma_start(
                        out=xn[sc][:, 0:2048], in_=x_d[:, 4096 * sc:mid])
                    nc.scalar.dma_start(
                        out=xn[sc][:, 2048:4096], in_=x_d[:, mid:4096 * (sc + 1)])
                else:
                    nc.sync.dma_start(
                        out=xn[sc][:, :], in_=x_d[:, 4096 * sc:4096 * (sc + 1)])
            def issue_w(nm, wd):
                nc.scalar.dma_start(out=wsb[nm][:, :], in_=wd[:, :])
            issue_xn(0)
            issue_w("wk", wk_d)
            nc.sync.dma_start(
                out=xn[1][:, :], in_=x_d[:, 4096:8192])
            issue_w("wq", wq_d)
            issue_w("wv", wv_d)
            for hp in range(NHP):
                nc.scalar.dma_start(
                    out=wp_sb[hp], in_=wp_d[P * hp:P * (hp + 1), :])

            with tc.tile_pool(name="sT_ps", bufs=2, space="PSUM") as sT_ps, \
                 tc.tile_pool(name="oT_ps", bufs=2, space="PSUM") as oT_ps, \
                 tc.tile_pool(name="fill_ps", bufs=2, space="PSUM") as fill_ps, \
                 tc.tile_pool(name="pT", bufs=8) as pT_pool, \
                 tc.tile_pool(name="dr", bufs=6) as dr_pool, \
                 tc.tile_pool(name="ysb", bufs=4) as y_pool:

                # ---------- PE clock priming ----------
                # The PE p-state ramps only after ~3us of continuous busy
                # (0.65 -> 1.2 -> 2.4 GHz).  The first ~5.7us are DMA-bound
                # with the PE idle, so the whole startup (transposes + qk
                # chains, ~10us of work) runs at half clock.  Dummy ident
                # transposes from t~0.7us (ident is the first DMA) keep the
                # PE busy through the DMA wait so real work starts hot.
                def prime(n):
                    t = sT_ps.tile([P, 1024], F32, tag="sT",
                                   name="sT").bitcast(F16)
                    for k in range(n):
                        nc.tensor.transpose(
                            t[:, P * (k % 4):P * (k % 4 + 1)], ident[:],
                            ident[:])

                # ---------- filler jobs (dependency-free PE work) ----------
                def tp_pair(sc, ep, pre=False, alt=False):
                    """transpose e-chunks 2ep, 2ep+1 of s-chunk sc into xT.
                    Shares the fill ring via bitcast (psum is bank-budgeted);
                    startup jobs alternate with the (then-idle) sT pool so
                    the 2-buf ring WAR doesn't gate the PE."""
                    if alt:
                        t = sT_ps.tile([P, 1024], F32, tag="sT",
                                       name="sT").bitcast(F16)
                    else:
                        t = fill_ps.tile([P, 512], F32, tag="fill",
                                         name="fill").bitcast(F16)
                    for j in range(2):
                        ec = 2 * ep + j
                        for k in range(4):
                            nc.tensor.transpose(
                                t[:, 512 * j + P * k:512 * j + P * (k + 1)],
                                xn[sc][:, 1024 * k + P * ec:1024 * k + P * (ec + 1)],
                                ident[:])
                    # psum->sbuf drains split between DVE and Scalar during
                    # the exp-free startup (GpSimd cannot read PSUM): a
                    # single vector queue serializes behind the 2-buf fill
                    # ring and gates the PE.
                    nc.vector.tensor_copy(
                        xT[2 * ep][:, 512 * sc:512 * (sc + 1)], t[:, 0:512])
                    if pre:
                        nc.scalar.copy(
                            xT[2 * ep + 1][:, 512 * sc:512 * (sc + 1)],
                            t[:, 512:1024])
                    else:
                        nc.vector.tensor_copy(
                            xT[2 * ep + 1][:, 512 * sc:512 * (sc + 1)],
                            t[:, 512:1024])

                def qk_chain(nm, hp, sc, pre=False, alt=False):
                    if alt:
                        ps = sT_ps.tile([P, 1024], F32, tag="sT",
                                        name="sT")[:, 0:512]
                    else:
                        ps = fill_ps.tile([P, 512], F32, tag="fill", name="fill")
                    for ec in range(NEC):
                        nc.tensor.matmul(
                            ps[:],
                            wsb[nm][:, HD * ec + P * hp:HD * ec + P * (hp + 1)],
                            xT[ec][:, 512 * sc:512 * (sc + 1)],
                            start=(ec == 0), stop=(ec == NEC - 1),
                        )
                    dest = qT if nm == "wq" else kT
                    if pre:
                        nc.scalar.copy(
                            dest[hp][:, 512 * sc:512 * (sc + 1)], ps[:])
                    else:
                        nc.vector.tensor_copy(
                            dest[hp][:, 512 * sc:512 * (sc + 1)], ps[:])

                def v_tile(st):
                    ps = fill_ps.tile([P, 512], F32, tag="fill", name="fill")
                    for ec in range(NEC):
                        nc.tensor.matmul(
                            ps[:, 0:HD],
                            xT[ec][:, P * st:P * (st + 1)],
                            wsb["wv"][:, HD * ec:HD * (ec + 1)],
                            start=(ec == 0), stop=(ec == NEC - 1),
                        )
                    nc.vector.tensor_copy(
                        v_view[:, st, :, D:P],
                        ps[:, 0:HD].rearrange("p (h c) -> p h c", h=NH),
                    )

                ysb_store = {}

                def proj_half(qt, nkk, on_scalar=False):
                    ps = fill_ps.tile([P, 512], F32, tag="fill", name="fill")
                    for hp in range(NHP):
                        nc.tensor.matmul(
                            ps[:],
                            oT_all[hp][:, P * qt:P * (qt + 1)],
                            wp_sb[hp][:, 512 * nkk:512 * (nkk + 1)],
                            start=(hp == 0), stop=(hp == NHP - 1),
                        )
                    ysb = ysb_store[qt]
                    if on_scalar:
                        nc.scalar.copy(ysb[:, 512 * nkk:512 * (nkk + 1)], ps[:])
                    else:
                        nc.vector.tensor_copy(
                            ysb[:, 512 * nkk:512 * (nkk + 1)], ps[:])
                    if nkk == 1:
                        # y rides the sync queue only: a scalar-queue trigger
                        # would make its sem-wait stall the exp stream.
                        nc.sync.dma_start(out=y_d[P * qt:P * (qt + 1), :], in_=ysb)

                def proj_jobs(qts, mix=False):
                    # mix: nkk0 cast on scalar (legal only where no exps
                    # remain on the scalar queue, i.e. the deferred-PV 0b)
                    jobs = []
                    for qt in qts:
                        ysb_store[qt] = y_pool.tile([P, E], F16, tag="ysb",
                                                    name=f"ysb{qt}")
                        jobs.append(lambda qt=qt: proj_half(qt, 0, on_scalar=mix))
                        jobs.append(lambda qt=qt: proj_half(qt, 1))
                    return jobs

                # ---------- attention pipeline ----------
                def attn_pipeline(blocks):
                    """blocks: list of (q0, qw, fillers, pop_n).  All (block,
                    hp, js) groups run as ONE flat software pipeline: the
                    2-group score lookahead crosses hp and block boundaries,
                    so neither has an S-emit bubble.  Filler legality is by
                    position: a job must sit early enough in its block's list
                    that everything depending on it (a later block's S via
                    lookahead, its own block's PV via v tiles) comes after
                    it in PE program order."""
                    njs = [(q0 + qw) // P for (q0, qw, _, _) in blocks]
                    seq = [(bi, hp, js)
                           for bi in range(len(blocks))
                           for hp in range(NHP)
                           for js in range(njs[bi])]
                    sT, pT, oT2s = {}, {}, {}
                    fill_i = [0] * len(blocks)

                    def pop_fillers(bi):
                        fl = blocks[bi][2]
                        for _ in range(blocks[bi][3]):
                            if fill_i[bi] < len(fl):
                                fl[fill_i[bi]]()
                                fill_i[bi] += 1

                    def flush(bi):
                        fl = blocks[bi][2]
                        while fill_i[bi] < len(fl):
                            fl[fill_i[bi]]()
                            fill_i[bi] += 1

                    def emit_S(bi, hp, js):
                        q0, qw, _, _ = blocks[bi]
                        cm = max(0, P * js - q0)
                        t = sT_ps.tile([P, 1024], F32, tag="sT", name="sT")
                        sT[bi, hp, js] = (t, cm)
                        for h in range(2):
                            lo = D * h
                            nc.tensor.matmul(
                                t[:, 512 * h + cm:512 * h + qw],
                                kT[hp][lo:lo + D, P * js:P * (js + 1)],
                                qT[hp][lo:lo + D, q0 + cm:q0 + qw],
                                start=True, stop=True,
                                tile_position=(lo, 0),
                            )

                    def emit_exp_mask(bi, hp, js):
                        # pT mirrors the psum layout (head h at 512h), so
                        # one exp spans both heads; the dead middle
                        # [qw, 512+cm) holds exp(garbage) and is never
                        # read.  One affine_select masks both heads via a
                        # zero-step h dimension.
                        q0, qw, _, _ = blocks[bi]
                        t, cm = sT[bi, hp, js]
                        p = pT_pool.tile([P, 1024], F16, tag="pT", name="pT")
                        pT[bi, hp, js] = (p, cm)
                        # one wide exp: ~209ns fixed cost per ACT instr
                        # makes per-head splitting a scalar-throughput
                        # loss even though it would halve the latency.
                        if qw <= 256:
                            for h in range(2):
                                nc.scalar.activation(
                                    p[:, 512 * h + cm:512 * h + qw],
                                    t[:, 512 * h + cm:512 * h + qw],
                                    mybir.ActivationFunctionType.Exp,
                                    scale=0.125)
                        else:
                            nc.scalar.activation(
                                p[:, cm:512 + qw], t[:, cm:512 + qw],
                                mybir.ActivationFunctionType.Exp, scale=0.125)
                        ce = min(cm + P, qw)
                        if P * js + P > q0:  # diagonal tile: causal mask
                            w = ce - cm
                            pv = p.rearrange("p (h c) -> p h c", h=2)
                            nc.gpsimd.affine_select(
                                out=pv[:, :, cm:ce],
                                in_=pv[:, :, cm:ce],
                                pattern=[[0, 2], [1, w]],
                                compare_op=mybir.AluOpType.is_ge,
                                fill=0.0,
                                base=q0 + cm - P * js,
                                channel_multiplier=-1,
                            )

                    def emit_PV(bi, hp, js):
                        q0, qw, _, _ = blocks[bi]
                        p, cm = pT.pop((bi, hp, js))
                        sT.pop((bi, hp, js))
                        oT2 = oT2s[bi, hp]
                        for h in range(2):
                            hl = 2 * hp + h
                            nc.tensor.matmul(
                                oT2[h][:, cm:qw],
                                v_view[:, js, hl, :],
                                p[:, 512 * h + cm:512 * h + qw],
                                start=(js == 0), stop=(js == njs[bi] - 1),
                            )

                    def norm(bi, hp):
                        # normalize: oT2 rows 0:64 = denominator (ones cols),
                        # rows 64:128 = numerator, per 512-half per head.
                        q0, qw, _, _ = blocks[bi]
                        oT2 = oT2s.pop((bi, hp))
                        for h in range(2):
                            dr = dr_pool.tile([D, 512], F32, tag="dr", name="dr")
                            nc.vector.reciprocal_approx_fast(
                                dr[:, 0:qw], oT2[h][0:D, 0:qw])
                            nc.vector.tensor_mul(
                                oT_all[hp][D * h:D * (h + 1), q0:q0 + qw],
                                oT2[h][D:P, 0:qw], dr[:, 0:qw])

                    last_bi = len(blocks) - 1
                    emit_S(*seq[0])
                    emit_S(*seq[1])
                    prev_bi = 0
                    for g, (bi, hp, js) in enumerate(seq):
                        if bi != prev_bi:
                            flush(prev_bi)
                            prev_bi = bi
                            if bi == last_bi:
                                # deferred-PV mode: the last block's S/exp/
                                # mask all emit upfront (pT holds 8 tiles),
                                # so its PVs, norms and proj fillers carry no
                                # scalar dependencies: proj casts can split
                                # onto the idle scalar queue and the final
                                # norm isn't stuck behind them on Vector.
                                lseq = [(bi, h2, j2) for h2 in range(NHP)
                                        for j2 in range(njs[bi])]
                                emit_S(*lseq[0])
                                emit_S(*lseq[1])
                                for k2, (b2, h2, j2) in enumerate(lseq):
                                    emit_exp_mask(b2, h2, j2)
                                    if k2 + 2 < len(lseq):
                                        emit_S(*lseq[k2 + 2])
                        if js == 0:
                            oT2s[bi, hp] = [
                                oT_ps.tile([P, 512], F32, tag="oT", name="oT")
                                for _ in range(2)]
                        if bi != last_bi:
                            emit_exp_mask(bi, hp, js)
                            if g + 2 < len(seq) and seq[g + 2][0] != last_bi:
                                emit_S(*seq[g + 2])
                        emit_PV(bi, hp, js)
                        if js == njs[bi] - 1:
                            norm(bi, hp)
                        pop_fillers(bi)
                    flush(last_bi)

                def tp_single(sc, ec):
                    """transpose one e-chunk of s-chunk sc (half a tp_pair)."""
                    t = fill_ps.tile([P, 512], F32, tag="fill",
                                     name="fill").bitcast(F16)
                    for k in range(4):
                        nc.tensor.transpose(
                            t[:, P * k:P * (k + 1)],
                            xn[sc][:, 1024 * k + P * ec:1024 * k + P * (ec + 1)],
                            ident[:])
                    nc.vector.tensor_copy(
                        xT[ec][:, 512 * sc:512 * (sc + 1)], t[:, 0:512])

                def TP(sc, ep):
                    return lambda: tp_pair(sc, ep)

                def TPS(sc, ec):
                    return lambda: tp_single(sc, ec)

                def QK(nm, hp, sc):
                    return lambda: qk_chain(nm, hp, sc)

                def V(st):
                    return lambda: v_tile(st)

                def XNV(sc, st):
                    def job():  # DMA issue rides a real PE job: no empty slot
                        issue_xn(sc)
                        v_tile(st)
                    return job

                # ---------- schedule ----------
                # pre-0a: transposes sc0, qk chains sc0 (with sc1 transposes
                # interleaved to hide the fill-copy latency), v0..v2
                prime(30)
                for ep in range(4):
                    tp_pair(0, ep, pre=True, alt=(ep % 2 == 1))
                qk_chain("wk", 0, 0, pre=True)
                tp_pair(1, 0, pre=True, alt=True)
                qk_chain("wk", 1, 0, pre=True, alt=False)
                tp_pair(1, 1, pre=True, alt=True)
                qk_chain("wq", 0, 0, alt=False)
                tp_pair(1, 2, pre=True, alt=True)
                qk_chain("wq", 1, 0, alt=False)
                tp_pair(1, 3, alt=True)
                v_tile(0)
                v_tile(1)
                v_tile(2)

                # Filler position constraints (cross-block S lookahead):
                # - the next block's qT chain for hp0 must pop >= 2 groups
                #   before its block starts (S emits 2 groups early);
                # - V(st) must pop before its j-tile's own-block PV;
                # - proj(qt) must pop after qt's block normed.
                # block 0a (q 0..384, 6 groups): wq-sc1-hp0 FIRST (b1's
                # lookahead S needs it by group 4)
                f0a = [QK("wq", 0, 1), QK("wk", 0, 1), QK("wk", 1, 1),
                       QK("wq", 1, 1), V(3)]

                # block 1 (q 512..1024, 16 groups): xn2 issue fused with v4,
                # v5..7 (own j-tiles), transposes sc2, qk(sc2) with wq-hp0
                # by position 13, v8,9 spill to the boundary flush.
                f1 = [XNV(2, 4), V(5), V(6), V(7)] + [
                      TPS(2, ec) for ec in range(NEC)] + [
                      QK("wk", 0, 2), QK("wq", 0, 2), QK("wk", 1, 2),
                      QK("wq", 1, 2), V(8), V(9)]

                # block 2 (q 1024..1536, 24 groups): xn3+v10, v11, transposes
                # sc3, proj(qt0,1), qk(sc3), proj(qt4,5)
                pj01 = proj_jobs([0, 1])
                pj45 = proj_jobs([4, 5])
                f2 = [XNV(3, 10), V(11)] + pj01[0:2] + [
                      TPS(3, ec) for ec in range(NEC)] + pj01[2:4] + [
                      QK("wk", 0, 3), QK("wq", 0, 3), QK("wk", 1, 3),
                      QK("wq", 1, 3)] + pj45

                # block 3 (q 1536..2048, 32 groups): v12..15 early, then
                # projections for ready columns (qt2 from 0a, 6,7 from b1)
                f3 = [V(12), V(13), V(14), V(15)]
                f3 += proj_jobs([2])

                # block 0b (q 384..512, 8 groups): proj(qt6..15) 3 per
                # group — 0b's groups are latency-bound, so they absorb
                # filler PE work that would extend the already-saturated b3.
                f0b = proj_jobs([6, 7, 8, 9, 10, 11, 12, 13, 14, 15],
                                mix=True)

                attn_pipeline([
                    (0, 384, f0a, 1),
                    (512, 512, f1, 1),
                    (1024, 512, f2, 1),
                    (1536, 512, f3, 1),
                    (384, 128, f0b, 3),
                ])

                # tail: qt3 only — both halves in one sT tile (attention is
                # done, the pool is free), copies split scalar/vector
                for qt in (3,):
                    ysb = y_pool.tile([P, E], F16, tag="ysb", name=f"ysb{qt}")
                    t = sT_ps.tile([P, 1024], F32, tag="sT", name="sT")
                    # hp-outer: both oT_all[0] halves run while norm(hp1) of
                    # the last block is still draining on Vector.
                    for hp in range(NHP):
                        for nkk in range(2):
                            nc.tensor.matmul(
                                t[:, 512 * nkk:512 * (nkk + 1)],
                                oT_all[hp][:, P * qt:P * (qt + 1)],
                                wp_sb[hp][:, 512 * nkk:512 * (nkk + 1)],
                                start=(hp == 0), stop=(hp == NHP - 1),
                            )
                    for nkk in range(2):
                        src = t[:, 512 * nkk:512 * (nkk + 1)]
                        dst = ysb[:, 512 * nkk:512 * (nkk + 1)]
                        # split across scalar and vector so they overlap
                        if nkk == 0:
                            nc.scalar.copy(dst, src)
                        else:
                            nc.vector.tensor_copy(dst, src)
                        eng = nc.sync if nkk == 0 else nc.scalar
                        eng.dma_start(
                            out=y_d[P * qt:P * (qt + 1), 512 * nkk:512 * (nkk + 1)],
                            in_=dst)

    if lower_isa:
        nc.finalize()
    return nc


_CACHED_NC = None


def _get_nc():
    global _CACHED_NC
    if _CACHED_NC is None:
        _CACHED_NC = build_core_program()
    return _CACHED_NC


def _pack_x(xb16):
    """[S, E] -> [P, NSC*4*E]: element [p, sc, k, e] = x[512*sc + 128*k + p, e]."""
    return np.ascontiguousarray(
        xb16.reshape(NSC, 4, P, E).transpose(2, 0, 1, 3).reshape(P, NSC * 4 * E))


def _pack_w(w16):
    """[E, HD] -> [P, NEC*HD]: element [p, ec, n] = w[128*ec + p, n]."""
    return np.ascontiguousarray(
        w16.reshape(NEC, P, HD).transpose(1, 0, 2).reshape(P, NEC * HD))


def shard_inputs(x, Wq, Wk, Wv, Wp):
    in_maps = []
    x16 = [_pack_x(np.asarray(x[b], dtype=np.float16)) for b in range(B)]
    wq16 = np.asarray(Wq, dtype=np.float16)
    wk16 = np.asarray(Wk, dtype=np.float16)
    wv16 = np.asarray(Wv, dtype=np.float16)
    wp16 = np.asarray(Wp, dtype=np.float16)
    ident = np.eye(P, dtype=np.float16)
    for core in range(N_CORES):
        b, g = core // GROUPS, core % GROUPS
        sl = slice(HD * g, HD * (g + 1))
        in_maps.append({
            "x": x16[b],
            "wq": _pack_w(wq16[:, sl]),
            "wk": _pack_w(wk16[:, sl]),
            "wv": _pack_w(wv16[:, sl]),
            "wp": np.ascontiguousarray(wp16[sl, :]),
            "ident": ident,
        })
    return in_maps


def _ensure_ntff_hook():
    """Provide antenv.axon_hooks (missing in this image) so trace=True can
    collect NTFF profiles through libaxon_pjrt's nrt-profile C ABI."""
    import types
    try:
        from antenv.axon_hooks import get_axon_ntff_profile_hook  # noqa: F401
        return
    except ImportError:
        pass
    import antenv
    mod = types.ModuleType("antenv.axon_hooks")
    mod._hook = None
    def set_axon_ntff_profile_hook(h):
        mod._hook = h
    def get_axon_ntff_profile_hook():
        return mod._hook
    mod.set_axon_ntff_profile_hook = set_axon_ntff_profile_hook
    mod.get_axon_ntff_profile_hook = get_axon_ntff_profile_hook
    sys.modules["antenv.axon_hooks"] = mod
    antenv.axon_hooks = mod
    try:
        from trn_agent_boot.trn_boot import _ntff_profile_via_ctypes
        mod._hook = _ntff_profile_via_ctypes("/opt/axon/libaxon_pjrt.so")
    except Exception as e:  # degrade: tracing skipped, run still works
        print(f"ntff hook setup failed: {e}", file=sys.stderr)


def run(inputs, trace=False, **spmd_kwargs):
    """Returns (full_output [B,S,E], BassKernelResults)."""
    from concourse.bass_utils import run_bass_kernel_spmd
    if trace:
        _ensure_ntff_hook()
    x = np.asarray(inputs["x"], dtype=np.float32)
    Wq = np.asarray(inputs["Wq"], dtype=np.float32)
    Wk = np.asarray(inputs["Wk"], dtype=np.float32)
    Wv = np.asarray(inputs["Wv"], dtype=np.float32)
    Wp = np.asarray(inputs["Wp"], dtype=np.float32)
    bp = np.asarray(inputs["bp"], dtype=np.float32)

    nc = _get_nc()
    in_maps = shard_inputs(x, Wq, Wk, Wv, Wp)
    res = run_bass_kernel_spmd(nc, in_maps, list(range(N_CORES)),
                               trace=trace, **spmd_kwargs)
    out = np.zeros((B, S, E), dtype=np.float32)
    for core in range(N_CORES):
        out[core // GROUPS] += res.results[core]["y"].astype(np.float32)
    out += bp[None, None, :]
    return out, res


def kernel(x, Wq, Wk, Wv, Wp, bp):
    out, _ = run({"x": x, "Wq": Wq, "Wk": Wk, "Wv": Wv, "Wp": Wp, "bp": bp})
    return out



# revision 35
# speedup vs baseline: 1.0133x; 1.0133x over previous
"""Causal multi-head attention (B=2, S=2048, E=1024, H=16, D=64) on 8 trn2 NeuronCores.

Sharding: core c handles batch b = c // 4 and head group g = c % 4 (4 heads each).
Each core computes, for its batch and its 4 heads:
    q/k/v = x @ W[qkv][:, 256g:256g+256], causal attention, then the partial
    projection  out_heads @ Wp[256g:256g+256, :]  -> [2048, 1024].
Host gathers: out[b] = sum_g partial[b, g] + bp  (the "all-reduce" of the TP hint).

fp16 end-to-end (inputs cast on host; tolerance is 2e-2, fp16 lands ~7e-4).
fp8 was evaluated and rejected: plain e4m3 on any main-path operand measures
2.3-2.9e-2 end-to-end (over the gate), and exact hi/lo-split variants cost
>= fp16 because DoubleRow halves the output rows (M<=64).

  - x and wq/wk/wv arrive HOST-PREPACKED in the exact SBUF layouts so each
    is ONE dma_start with 4-8KB contiguous lines; s-chunk 0 is split across
    the two HWDGE queues (sync+scalar, ~180GB/s each).  dma_start costs
    ~1.5us fixed (trigger + sem), so fewer/bigger transfers win.
  - the PE p-state ramps 0.65 -> 1.2 -> 2.4 GHz only after ~3us of
    continuous busy; ~30 dummy ident transposes bridge the initial DMA wait
    so real work starts at full clock.
  - x is PE-transposed (1 cyc/row fp16) into xT e-chunks; startup psum
    drains are split vector/scalar and alternate fill/sT pools so the 2-buf
    ring WAR never gates the PE.
  - kT/qT are [head-pair, S] tiles with the pair's heads stacked at
    partition 0/64; scores use tile_position=(64h, 0).
  - v is stored per (s-tile, head) as a [128, 128] slab [ones64 | v64]; the
    PV matmul then emits the softmax denominator on partitions 0..63 and the
    numerator on 64..127, so normalization is reciprocal+mul on the vector
    engine (no partition_broadcast, no copies).
  - ALL attention groups (block, head-pair, j-tile) run as ONE flat software
    pipeline (attn_pipeline): the 2-group score lookahead crosses hp and
    block boundaries, norms are emitted inline right after each hp's last
    PV, and filler jobs (qk chains / v tiles / transposes / projection
    halves) are popped per group to hide the scalar exp stream (~1.05us per
    512-wide group, the steady-state pacer).
  - each score matmul's PSUM region starts 512-f32 aligned: two matmul groups
    packed into one PSUM bank at sub-bank offsets fail at runtime.
  - q-blocks run in order [0,384) [512,1024) [1024,1536) [1536,2048) [384,512):
    the last block is 128 q wide, so the end tail (norm+proj+DMA) is short.
    0b's groups are latency-bound and absorb the proj fillers; 2 jobs are
    reserved past the last pop so the PE has work while the final norm
    drains on Vector, and the tail projection interleaves its two psum
    chains hp-outer so the oT_all[0] halves run during norm(hp1).
"""

import os
import sys
import numpy as np

sys.path.insert(0, "/opt/trn_rl_repo")

import concourse.bass as bass
import concourse.bacc as bacc_mod
import concourse.mybir as mybir
import concourse.tile as tile
from concourse import library_config

F32 = mybir.dt.float32
F16 = mybir.dt.float16
P = 128

B = 2
S = 2048
E = 1024
NHEADS_TOTAL = 16
D = 64
N_CORES = 8
GROUPS = 4                        # head groups (tensor parallel)
HD = NHEADS_TOTAL * D // GROUPS   # 256 head-dims per core
NH = HD // D                      # heads per core (4)
NHP = HD // P                     # head pairs (2)
NST = S // P                      # s tiles (16)
NEC = E // P                      # e chunks (8)
NSC = S // 512                    # 512-wide s chunks (4)


def build_core_program(lower_isa=True):
    """One NeuronCore's program (SPMD: all 8 cores run this on different data)."""
    nc = bacc_mod.Bacc()
    # x and wq/wk/wv arrive HOST-PREPACKED in the exact SBUF layouts
    # (x: [p, sc, k, e], w: [p, ec, n]) so every DMA line is 4-8KB
    # contiguous and each tensor is ONE dma_start: the startup was
    # descriptor/trigger-latency bound (~1.5us fixed per dma_start,
    # 2x bandwidth penalty for <512B lines).
    x_d = nc.declare_dram_parameter("x", [P, NSC * 4096], F16, False)
    wq_d = nc.declare_dram_parameter("wq", [P, NEC * HD], F16, False)
    wk_d = nc.declare_dram_parameter("wk", [P, NEC * HD], F16, False)
    wv_d = nc.declare_dram_parameter("wv", [P, NEC * HD], F16, False)
    wp_d = nc.declare_dram_parameter("wp", [HD, E], F16, False)
    # identity comes in as data: building it with gpsimd memset+affine_select
    # would serialize the first PE transpose behind the ~10us gpsimd library
    # load DMA.
    id_d = nc.declare_dram_parameter("ident", [P, P], F16, False)
    y_d = nc.declare_dram_parameter("y", [S, E], F16, True)

    with tile.TileContext(nc) as tc:
        from contextlib import ExitStack
        with ExitStack() as ctx:
            persist = ctx.enter_context(tc.tile_pool(name="persist", bufs=1))

            ident = persist.tile([P, P], F16, tag="ident", name="ident")

            xT = [persist.tile([P, S], F16, tag=f"xT{ec}", name=f"xT{ec}")
                  for ec in range(NEC)]
            xn = [persist.tile([P, 4096], F16, tag=f"xn{sc}", name=f"xn{sc}")
                  for sc in range(NSC)]
            wsb = {nm: persist.tile([P, NEC * HD], F16, tag=nm, name=nm)
                   for nm in ("wq", "wk", "wv")}
            wp_sb = [persist.tile([P, E], F16, tag=f"wp{hp}", name=f"wp{hp}")
                     for hp in range(NHP)]
            qT = [persist.tile([P, S], F16, tag=f"qT{hp}", name=f"qT{hp}")
                  for hp in range(NHP)]
            kT = [persist.tile([P, S], F16, tag=f"kT{hp}", name=f"kT{hp}")
                  for hp in range(NHP)]
            # per (s-tile, head) slab [128, 128] = [ones 0:64 | v 64:128]
            v_ext = persist.tile([P, NST * NH * P], F16, tag="v_ext", name="v_ext")
            oT_all = [persist.tile([P, S], F16, tag=f"oT{hp}", name=f"oT{hp}")
                      for hp in range(NHP)]

            v_view = v_ext.rearrange("p (s h c) -> p s h c", s=NST, h=NH)
            nc.vector.memset(
                v_ext.rearrange("p (s c) -> p s c", s=NST * NH)[:, :, 0:D], 1.0)

            # ---------------- DMA issue ----------------
            # Prepacked loads with 4-8KB lines.  Only sync (SP) and scalar
            # (Activation) queues can trigger HWDGE; each sustains ~180GB/s,
            # so s-chunk 0 is split across BOTH queues and the rest is
            # deadline-ordered: sync [ident, x0a, x1], scalar [x0b, wk, wq,
            # wv, wp].  x chunks 2/3 are deferred (XNV filler jobs), split
            # across both queues.
            nc.sync.dma_start(out=ident[:], in_=id_d[:, :])
            def issue_xn(sc, split=True):
                mid = 4096 * sc + 2048
                if split:
                    nc.sync.dma_start(
                        out=xn[sc][:, 0:2048], in_=x_d[:, 4096 * sc:mid])
                    nc.scalar.dma_start(
                        out=xn[sc][:, 2048:4096], in_=x_d[:, mid:4096 * (sc + 1)])
                else:
                    nc.sync.dma_start(
                        out=xn[sc][:, :], in_=x_d[:, 4096 * sc:4096 * (sc + 1)])
            def issue_w(nm, wd):
                nc.scalar.dma_start(out=wsb[nm][:, :], in_=wd[:, :])
            issue_xn(0)
            issue_w("wk", wk_d)
            nc.sync.dma_start(
                out=xn[1][:, :], in_=x_d[:, 4096:8192])
            issue_w("wq", wq_d)
            issue_w("wv", wv_d)
            for hp in range(NHP):
                nc.scalar.dma_start(
                    out=wp_sb[hp], in_=wp_d[P * hp:P * (hp + 1), :])

            with tc.tile_pool(name="sT_ps", bufs=2, space="PSUM") as sT_ps, \
                 tc.tile_pool(name="oT_ps", bufs=2, space="PSUM") as oT_ps, \
                 tc.tile_pool(name="fill_ps", bufs=2, space="PSUM") as fill_ps, \
                 tc.tile_pool(name="pT", bufs=6) as pT_pool, \
                 tc.tile_pool(name="dr", bufs=6) as dr_pool, \
                 tc.tile_pool(name="ysb", bufs=4) as y_pool:

                # ---------- PE clock priming ----------
                # The PE p-state ramps only after ~3us of continuous busy
                # (0.65 -> 1.2 -> 2.4 GHz).  The first ~5.7us are DMA-bound
                # with the PE idle, so the whole startup (transposes + qk
                # chains, ~10us of work) runs at half clock.  Dummy ident
                # transposes from t~0.7us (ident is the first DMA) keep the
                # PE busy through the DMA wait so real work starts hot.
                def prime(n):
                    t = sT_ps.tile([P, 1024], F32, tag="sT",
                                   name="sT").bitcast(F16)
                    for k in range(n):
                        nc.tensor.transpose(
                            t[:, P * (k % 4):P * (k % 4 + 1)], ident[:],
                            ident[:])

                # ---------- filler jobs (dependency-free PE work) ----------
                def tp_pair(sc, ep, pre=False, alt=False):
                    """transpose e-chunks 2ep, 2ep+1 of s-chunk sc into xT.
                    Shares the fill ring via bitcast (psum is bank-budgeted);
                    startup jobs alternate with the (then-idle) sT pool so
                    the 2-buf ring WAR doesn't gate the PE."""
                    if alt:
                        t = sT_ps.tile([P, 1024], F32, tag="sT",
                                       name="sT").bitcast(F16)
                    else:
                        t = fill_ps.tile([P, 512], F32, tag="fill",
                                         name="fill").bitcast(F16)
                    for j in range(2):
                        ec = 2 * ep + j
                        for k in range(4):
                            nc.tensor.transpose(
                                t[:, 512 * j + P * k:512 * j + P * (k + 1)],
                                xn[sc][:, 1024 * k + P * ec:1024 * k + P * (ec + 1)],
                                ident[:])
                    # psum->sbuf drains split between DVE and Scalar during
                    # the exp-free startup (GpSimd cannot read PSUM): a
                    # single vector queue serializes behind the 2-buf fill
                    # ring and gates the PE.
                    nc.vector.tensor_copy(
                        xT[2 * ep][:, 512 * sc:512 * (sc + 1)], t[:, 0:512])
                    if pre:
                        nc.scalar.copy(
                            xT[2 * ep + 1][:, 512 * sc:512 * (sc + 1)],
                            t[:, 512:1024])
                    else:
                        nc.vector.tensor_copy(
                            xT[2 * ep + 1][:, 512 * sc:512 * (sc + 1)],
                            t[:, 512:1024])

                def qk_chain(nm, hp, sc, pre=False, alt=False):
                    if alt:
                        ps = sT_ps.tile([P, 1024], F32, tag="sT",
                                        name="sT")[:, 0:512]
                    else:
                        ps = fill_ps.tile([P, 512], F32, tag="fill", name="fill")
                    for ec in range(NEC):
                        nc.tensor.matmul(
                            ps[:],
                            wsb[nm][:, HD * ec + P * hp:HD * ec + P * (hp + 1)],
                            xT[ec][:, 512 * sc:512 * (sc + 1)],
                            start=(ec == 0), stop=(ec == NEC - 1),
                        )
                    dest = qT if nm == "wq" else kT
                    if pre:
                        nc.scalar.copy(
                            dest[hp][:, 512 * sc:512 * (sc + 1)], ps[:])
                    else:
                        nc.vector.tensor_copy(
                            dest[hp][:, 512 * sc:512 * (sc + 1)], ps[:])

                def v_tile(st):
                    ps = fill_ps.tile([P, 512], F32, tag="fill", name="fill")
                    for ec in range(NEC):
                        nc.tensor.matmul(
                            ps[:, 0:HD],
                            xT[ec][:, P * st:P * (st + 1)],
                            wsb["wv"][:, HD * ec:HD * (ec + 1)],
                            start=(ec == 0), stop=(ec == NEC - 1),
                        )
                    nc.vector.tensor_copy(
                        v_view[:, st, :, D:P],
                        ps[:, 0:HD].rearrange("p (h c) -> p h c", h=NH),
                    )

                ysb_store = {}

                def proj_half(qt, nkk):
                    ps = fill_ps.tile([P, 512], F32, tag="fill", name="fill")
                    for hp in range(NHP):
                        nc.tensor.matmul(
                            ps[:],
                            oT_all[hp][:, P * qt:P * (qt + 1)],
                            wp_sb[hp][:, 512 * nkk:512 * (nkk + 1)],
                            start=(hp == 0), stop=(hp == NHP - 1),
                        )
                    ysb = ysb_store[qt]
                    nc.vector.tensor_copy(ysb[:, 512 * nkk:512 * (nkk + 1)], ps[:])
                    if nkk == 1:
                        # y rides the sync queue only: a scalar-queue trigger
                        # would make its sem-wait stall the exp stream.
                        nc.sync.dma_start(out=y_d[P * qt:P * (qt + 1), :], in_=ysb)

                def proj_jobs(qts):
                    jobs = []
                    for qt in qts:
                        ysb_store[qt] = y_pool.tile([P, E], F16, tag="ysb",
                                                    name=f"ysb{qt}")
                        jobs.append(lambda qt=qt: proj_half(qt, 0))
                        jobs.append(lambda qt=qt: proj_half(qt, 1))
                    return jobs

                # ---------- attention pipeline ----------
                def attn_pipeline(blocks):
                    """blocks: list of (q0, qw, fillers, pop_n).  All (block,
                    hp, js) groups run as ONE flat software pipeline: the
                    2-group score lookahead crosses hp and block boundaries,
                    so neither has an S-emit bubble.  Filler legality is by
                    position: a job must sit early enough in its block's list
                    that everything depending on it (a later block's S via
                    lookahead, its own block's PV via v tiles) comes after
                    it in PE program order."""
                    njs = [(q0 + qw) // P for (q0, qw, _, _) in blocks]
                    seq = [(bi, hp, js)
                           for bi in range(len(blocks))
                           for hp in range(NHP)
                           for js in range(njs[bi])]
                    sT, pT, oT2s = {}, {}, {}
                    fill_i = [0] * len(blocks)

                    def pop_fillers(bi):
                        fl = blocks[bi][2]
                        for _ in range(blocks[bi][3]):
                            if fill_i[bi] < len(fl):
                                fl[fill_i[bi]]()
                                fill_i[bi] += 1

                    def flush(bi):
                        fl = blocks[bi][2]
                        while fill_i[bi] < len(fl):
                            fl[fill_i[bi]]()
                            fill_i[bi] += 1

                    def emit_S(bi, hp, js):
                        q0, qw, _, _ = blocks[bi]
                        cm = max(0, P * js - q0)
                        t = sT_ps.tile([P, 1024], F32, tag="sT", name="sT")
                        sT[bi, hp, js] = (t, cm)
                        for h in range(2):
                            lo = D * h
                            nc.tensor.matmul(
                                t[:, 512 * h + cm:512 * h + qw],
                                kT[hp][lo:lo + D, P * js:P * (js + 1)],
                                qT[hp][lo:lo + D, q0 + cm:q0 + qw],
                                start=True, stop=True,
                                tile_position=(lo, 0),
                            )

                    def emit_exp_mask(bi, hp, js):
                        # pT mirrors the psum layout (head h at 512h), so
                        # one exp spans both heads; the dead middle
                        # [qw, 512+cm) holds exp(garbage) and is never
                        # read.  One affine_select masks both heads via a
                        # zero-step h dimension.
                        q0, qw, _, _ = blocks[bi]
                        t, cm = sT[bi, hp, js]
                        p = pT_pool.tile([P, 1024], F16, tag="pT", name="pT")
                        pT[bi, hp, js] = (p, cm)
                        # one wide exp: ~209ns fixed cost per ACT instr
                        # makes per-head splitting a scalar-throughput
                        # loss even though it would halve the latency.
                        if qw <= 256:
                            for h in range(2):
                                nc.scalar.activation(
                                    p[:, 512 * h + cm:512 * h + qw],
                                    t[:, 512 * h + cm:512 * h + qw],
                                    mybir.ActivationFunctionType.Exp,
                                    scale=0.125)
                        else:
                            nc.scalar.activation(
                                p[:, cm:512 + qw], t[:, cm:512 + qw],
                                mybir.ActivationFunctionType.Exp, scale=0.125)
                        ce = min(cm + P, qw)
                        if P * js + P > q0:  # diagonal tile: causal mask
                            w = ce - cm
                            pv = p.rearrange("p (h c) -> p h c", h=2)
                            nc.gpsimd.affine_select(
                                out=pv[:, :, cm:ce],
                                in_=pv[:, :, cm:ce],
                                pattern=[[0, 2], [1, w]],
                                compare_op=mybir.AluOpType.is_ge,
                                fill=0.0,
                                base=q0 + cm - P * js,
                                channel_multiplier=-1,
                            )

                    def emit_PV(bi, hp, js):
                        q0, qw, _, _ = blocks[bi]
                        p, cm = pT.pop((bi, hp, js))
                        sT.pop((bi, hp, js))
                        oT2 = oT2s[bi, hp]
                        for h in range(2):
                            hl = 2 * hp + h
                            nc.tensor.matmul(
                                oT2[h][:, cm:qw],
                                v_view[:, js, hl, :],
                                p[:, 512 * h + cm:512 * h + qw],
                                start=(js == 0), stop=(js == njs[bi] - 1),
                            )

                    def norm(bi, hp):
                        # normalize: oT2 rows 0:64 = denominator (ones cols),
                        # rows 64:128 = numerator, per 512-half per head.
                        q0, qw, _, _ = blocks[bi]
                        oT2 = oT2s.pop((bi, hp))
                        for h in range(2):
                            dr = dr_pool.tile([D, 512], F32, tag="dr", name="dr")
                            nc.vector.reciprocal_approx_fast(
                                dr[:, 0:qw], oT2[h][0:D, 0:qw])
                            nc.vector.tensor_mul(
                                oT_all[hp][D * h:D * (h + 1), q0:q0 + qw],
                                oT2[h][D:P, 0:qw], dr[:, 0:qw])

                    emit_S(*seq[0])
                    emit_S(*seq[1])
                    prev_bi = 0
                    for g, (bi, hp, js) in enumerate(seq):
                        if bi != prev_bi:
                            flush(prev_bi)
                            prev_bi = bi
                        if js == 0:
                            oT2s[bi, hp] = [
                                oT_ps.tile([P, 512], F32, tag="oT", name="oT")
                                for _ in range(2)]
                        emit_exp_mask(bi, hp, js)
                        if g + 2 < len(seq):
                            emit_S(*seq[g + 2])
                        emit_PV(bi, hp, js)
                        if js == njs[bi] - 1:
                            norm(bi, hp)
                        pop_fillers(bi)
                    flush(len(blocks) - 1)

                def tp_single(sc, ec):
                    """transpose one e-chunk of s-chunk sc (half a tp_pair)."""
                    t = fill_ps.tile([P, 512], F32, tag="fill",
                                     name="fill").bitcast(F16)
                    for k in range(4):
                        nc.tensor.transpose(
                            t[:, P * k:P * (k + 1)],
                            xn[sc][:, 1024 * k + P * ec:1024 * k + P * (ec + 1)],
                            ident[:])
                    nc.vector.tensor_copy(
                        xT[ec][:, 512 * sc:512 * (sc + 1)], t[:, 0:512])

                def TP(sc, ep):
                    return lambda: tp_pair(sc, ep)

                def TPS(sc, ec):
                    return lambda: tp_single(sc, ec)

                def QK(nm, hp, sc):
                    return lambda: qk_chain(nm, hp, sc)

                def V(st):
                    return lambda: v_tile(st)

                def XNV(sc, st):
                    def job():  # DMA issue rides a real PE job: no empty slot
                        issue_xn(sc)
                        v_tile(st)
                    return job

                # ---------- schedule ----------
                # pre-0a: transposes sc0, qk chains sc0 (with sc1 transposes
                # interleaved to hide the fill-copy latency), v0..v2
                prime(30)
                for ep in range(4):
                    tp_pair(0, ep, pre=True, alt=(ep % 2 == 1))
                qk_chain("wk", 0, 0, pre=True)
                tp_pair(1, 0, pre=True, alt=True)
                qk_chain("wk", 1, 0, pre=True, alt=False)
                tp_pair(1, 1, pre=True, alt=True)
                qk_chain("wq", 0, 0, alt=False)
                tp_pair(1, 2, pre=True, alt=True)
                qk_chain("wq", 1, 0, alt=False)
                tp_pair(1, 3, alt=True)
                v_tile(0)
                v_tile(1)
                v_tile(2)

                # Filler position constraints (cross-block S lookahead):
                # - the next block's qT chain for hp0 must pop >= 2 groups
                #   before its block starts (S emits 2 groups early);
                # - V(st) must pop before its j-tile's own-block PV;
                # - proj(qt) must pop after qt's block normed.
                # block 0a (q 0..384, 6 groups): wq-sc1-hp0 FIRST (b1's
                # lookahead S needs it by group 4)
                f0a = [QK("wq", 0, 1), QK("wk", 0, 1), QK("wk", 1, 1),
                       QK("wq", 1, 1), V(3)]

                # block 1 (q 512..1024, 16 groups): xn2 issue fused with v4,
                # v5..7 (own j-tiles), transposes sc2, qk(sc2) with wq-hp0
                # by position 13, v8,9 spill to the boundary flush.
                f1 = [XNV(2, 4), V(5), V(6), V(7)] + [
                      TPS(2, ec) for ec in range(NEC)] + [
                      QK("wk", 0, 2), QK("wq", 0, 2), QK("wk", 1, 2),
                      QK("wq", 1, 2), V(8), V(9)]

                # block 2 (q 1024..1536, 24 groups): xn3+v10, v11, transposes
                # sc3, proj(qt0,1), qk(sc3), proj(qt4,5)
                pj01 = proj_jobs([0, 1])
                pj45 = proj_jobs([4, 5])
                f2 = [XNV(3, 10), V(11)] + pj01[0:2] + [
                      TPS(3, ec) for ec in range(NEC)] + pj01[2:4] + [
                      QK("wk", 0, 3), QK("wq", 0, 3), QK("wk", 1, 3),
                      QK("wq", 1, 3)] + pj45

                # block 3 (q 1536..2048, 32 groups): v12..15 early, then
                # projections for ready columns (qt2 from 0a, 6,7 from b1)
                f3 = [V(12), V(13), V(14), V(15)]
                f3 += proj_jobs([2])

                # block 0b (q 384..512, 8 groups): proj(qt6..15) 3 per
                # group — 0b's groups are latency-bound, so they absorb
                # filler PE work that would extend the already-saturated b3.
                f0b = proj_jobs([6, 7, 8, 9, 10, 11, 12, 13, 14, 15])

                attn_pipeline([
                    (0, 384, f0a, 1),
                    (512, 512, f1, 1),
                    (1024, 512, f2, 1),
                    (1536, 512, f3, 1),
                    (384, 128, f0b, 3),
                ])

                # tail: qt3 only — both halves in one sT tile (attention is
                # done, the pool is free), copies split scalar/vector
                for qt in (3,):
                    ysb = y_pool.tile([P, E], F16, tag="ysb", name=f"ysb{qt}")
                    t = sT_ps.tile([P, 1024], F32, tag="sT", name="sT")
                    # hp-outer: both oT_all[0] halves run while norm(hp1) of
                    # the last block is still draining on Vector.
                    for hp in range(NHP):
                        for nkk in range(2):
                            nc.tensor.matmul(
                                t[:, 512 * nkk:512 * (nkk + 1)],
                                oT_all[hp][:, P * qt:P * (qt + 1)],
                                wp_sb[hp][:, 512 * nkk:512 * (nkk + 1)],
                                start=(hp == 0), stop=(hp == NHP - 1),
                            )
                    for nkk in range(2):
                        src = t[:, 512 * nkk:512 * (nkk + 1)]
                        dst = ysb[:, 512 * nkk:512 * (nkk + 1)]
                        # split across scalar and vector so they overlap
                        if nkk == 0:
                            nc.scalar.copy(dst, src)
                        else:
                            nc.vector.tensor_copy(dst, src)
                        eng = nc.sync if nkk == 0 else nc.scalar
                        eng.dma_start(
                            out=y_d[P * qt:P * (qt + 1), 512 * nkk:512 * (nkk + 1)],
                            in_=dst)

    if lower_isa:
        nc.finalize()
    return nc


_CACHED_NC = None


def _get_nc():
    global _CACHED_NC
    if _CACHED_NC is None:
        _CACHED_NC = build_core_program()
    return _CACHED_NC


def _pack_x(xb16):
    """[S, E] -> [P, NSC*4*E]: element [p, sc, k, e] = x[512*sc + 128*k + p, e]."""
    return np.ascontiguousarray(
        xb16.reshape(NSC, 4, P, E).transpose(2, 0, 1, 3).reshape(P, NSC * 4 * E))


def _pack_w(w16):
    """[E, HD] -> [P, NEC*HD]: element [p, ec, n] = w[128*ec + p, n]."""
    return np.ascontiguousarray(
        w16.reshape(NEC, P, HD).transpose(1, 0, 2).reshape(P, NEC * HD))


def shard_inputs(x, Wq, Wk, Wv, Wp):
    in_maps = []
    x16 = [_pack_x(np.asarray(x[b], dtype=np.float16)) for b in range(B)]
    wq16 = np.asarray(Wq, dtype=np.float16)
    wk16 = np.asarray(Wk, dtype=np.float16)
    wv16 = np.asarray(Wv, dtype=np.float16)
    wp16 = np.asarray(Wp, dtype=np.float16)
    ident = np.eye(P, dtype=np.float16)
    for core in range(N_CORES):
        b, g = core // GROUPS, core % GROUPS
        sl = slice(HD * g, HD * (g + 1))
        in_maps.append({
            "x": x16[b],
            "wq": _pack_w(wq16[:, sl]),
            "wk": _pack_w(wk16[:, sl]),
            "wv": _pack_w(wv16[:, sl]),
            "wp": np.ascontiguousarray(wp16[sl, :]),
            "ident": ident,
        })
    return in_maps


def _ensure_ntff_hook():
    """Provide antenv.axon_hooks (missing in this image) so trace=True can
    collect NTFF profiles through libaxon_pjrt's nrt-profile C ABI."""
    import types
    try:
        from antenv.axon_hooks import get_axon_ntff_profile_hook  # noqa: F401
        return
    except ImportError:
        pass
    import antenv
    mod = types.ModuleType("antenv.axon_hooks")
    mod._hook = None
    def set_axon_ntff_profile_hook(h):
        mod._hook = h
    def get_axon_ntff_profile_hook():
        return mod._hook
    mod.set_axon_ntff_profile_hook = set_axon_ntff_profile_hook
    mod.get_axon_ntff_profile_hook = get_axon_ntff_profile_hook
    sys.modules["antenv.axon_hooks"] = mod
    antenv.axon_hooks = mod
    try:
        from trn_agent_boot.trn_boot import _ntff_profile_via_ctypes
        mod._hook = _ntff_profile_via_ctypes("/opt/axon/libaxon_pjrt.so")
    except Exception as e:  # degrade: tracing skipped, run still works
        print(f"ntff hook setup failed: {e}", file=sys.stderr)


def run(inputs, trace=False, **spmd_kwargs):
    """Returns (full_output [B,S,E], BassKernelResults)."""
    from concourse.bass_utils import run_bass_kernel_spmd
    if trace:
        _ensure_ntff_hook()
    x = np.asarray(inputs["x"], dtype=np.float32)
    Wq = np.asarray(inputs["Wq"], dtype=np.float32)
    Wk = np.asarray(inputs["Wk"], dtype=np.float32)
    Wv = np.asarray(inputs["Wv"], dtype=np.float32)
    Wp = np.asarray(inputs["Wp"], dtype=np.float32)
    bp = np.asarray(inputs["bp"], dtype=np.float32)

    nc = _get_nc()
    in_maps = shard_inputs(x, Wq, Wk, Wv, Wp)
    res = run_bass_kernel_spmd(nc, in_maps, list(range(N_CORES)),
                               trace=trace, **spmd_kwargs)
    out = np.zeros((B, S, E), dtype=np.float32)
    for core in range(N_CORES):
        out[core // GROUPS] += res.results[core]["y"].astype(np.float32)
    out += bp[None, None, :]
    return out, res


def kernel(x, Wq, Wk, Wv, Wp, bp):
    out, _ = run({"x": x, "Wq": Wq, "Wk": Wk, "Wv": Wv, "Wp": Wp, "bp": bp})
    return out

